# revision 1
# baseline (speedup 1.0000x reference)
"""Trainium2 Bass kernel for nn_ColumnStep (scatter_memory).

Contract: kernel(**inputs) takes FULL unsharded inputs (numpy-convertible),
returns the FULL (B, T, V) float32 output.

Sharding: 8 cores = B(2) x T-query-chunks(4). Each core holds the full
gathered sequence for its batch (keys/values of the anti-causal decay
attention) and computes a 512-row query chunk. Parameters are replicated.
Host does only the vocab gather / zero-scatter and layout prep; all
arithmetic runs on-device.

Everything is kept in transposed (k-major) layout on device so rmsnorm
reductions become ones-vector matmuls and no PE transposes are needed.
Large matmuls run with float32r operands (full-rate PE, ~1e-4 rel err).
"""

import sys

for _p in ("/opt/trn_rl_repo", "/root/.axon_site/_ro/trn_rl_repo"):
    if _p not in sys.path:
        sys.path.append(_p)

import numpy as np

import concourse.bass as bass  # noqa: F401  (registers engine mixins)
import concourse.mybir as mybir
from concourse import bacc, tile
from concourse.bass_utils import run_bass_kernel_spmd

F32 = mybir.dt.float32
F32R = mybir.dt.float32r
AF = mybir.ActivationFunctionType
OP = mybir.AluOpType

# Problem shape (hardcoded per spec)
V, K, B, T, NB, INNER = 32000, 256, 2, 2048, 4, 128
EPS = 1.1920929e-07
P = 128          # partitions
NT = T // P      # 16 full-sequence j tiles
QF = T // 4      # 512 query rows per core
NQ = QF // P     # 4 query tiles per core
KT = K // P      # 2 tiles along the k=256 dim
NC5 = T // 512   # 4 512-wide column chunks of the full sequence

_prog_cache = {}


def _build_program(s_qk, c_mem, s_out):
    """Build the SPMD Bass/Tile program. Scalars are baked as immediates."""
    nc = bacc.Bacc("TRN2", target_bir_lowering=False, debug=False, num_devices=8)

    gT_d = nc.dram_tensor("gT", [KT, P, T], F32, kind="ExternalInput")
    gqT_d = nc.dram_tensor("gqT", [KT, P, QF], F32, kind="ExternalInput")
    wd2_d = nc.dram_tensor("wd", [NT // 2, P, 2, QF], F32, kind="ExternalInput")
    # packed f32r params per partition: wall (4*KT*K) | bd (NB*KT*INNER)
    # | bu (NB*K) | gw (KT*NB) | ones_col (1)
    PK = 4 * KT * K + NB * KT * INNER + NB * K + KT * NB + 1
    pack_d = nc.dram_tensor("pack", [P, PK], F32R, kind="ExternalInput")
    onesc_d = nc.dram_tensor("onesc", [P, 1], F32R, kind="ExternalInput")
    biash_d = nc.dram_tensor("biash", [P, 1], F32, kind="ExternalInput")
    gateb_d = nc.dram_tensor("gateb", [P, NB], F32, kind="ExternalInput")
    onesr_d = nc.dram_tensor("onesr", [1, P], F32R, kind="ExternalInput")
    o_d = nc.dram_tensor("o", [NQ, P, K], F32, kind="ExternalOutput")

    WQ, WK, WV, WO = 0, 1, 2, 3
    AX = mybir.AxisListType.X

    with tile.TileContext(nc) as tc:
        with (
            tc.tile_pool(name="const", bufs=1) as cp,
            tc.tile_pool(name="persist", bufs=1) as pp,
            tc.tile_pool(name="work", bufs=3) as wp,
            tc.tile_pool(name="stat", bufs=4) as sp,
            tc.tile_pool(name="psM", bufs=4, space="PSUM") as psM,
            tc.tile_pool(name="psN", bufs=1, space="PSUM") as psN,
            tc.tile_pool(name="psR", bufs=1, space="PSUM") as psR,
        ):
            # ---- constants / parameters (packed; DMAs issued after gT) ----
            pack_t = cp.tile([P, PK], F32R, tag="pack")
            o1 = 4 * KT * K
            o2 = o1 + NB * KT * INNER
            o3 = o2 + NB * K
            o4 = o3 + KT * NB
            w_t = pack_t[:, 0:o1].rearrange("p (w t k) -> p w t k", w=4, t=KT)
            bd_t = pack_t[:, o1:o2].rearrange("p (n t h) -> p n t h", n=NB, t=KT)
            bu_t = pack_t[:, o2:o3].rearrange("p (n k) -> p n k", n=NB)
            gw_t = pack_t[:, o3:o4].rearrange("p (t n) -> p t n", t=KT)
            ones_col = cp.tile([P, 1], F32R, tag="ones_col")
            biash_t = cp.tile([P, 1], F32, tag="biash")
            gateb_t = cp.tile([P, NB], F32, tag="gateb")
            eps1_t = cp.tile([1, 1], F32, tag="eps1")
            nc.vector.memset(eps1_t[:], EPS)
            ones_row = cp.tile([1, P], F32R, tag="ones_row")  # broadcast lhsT

            # ---- persistent intermediates (k-major / transposed layouts) ----
            gT = [pp.tile([P, T], F32, tag=f"gT{i}", name=f"gT{i}") for i in range(KT)]
            gqT = [pp.tile([P, QF], F32, tag=f"gqT{i}", name=f"gqT{i}") for i in range(KT)]
            gnT = [pp.tile([P, T], F32R, tag=f"gnT{i}", name=f"gnT{i}") for i in range(KT)]
            gqnT = [pp.tile([P, QF], F32R, tag=f"gqnT{i}", name=f"gqnT{i}") for i in range(KT)]
            kkT = [pp.tile([P, T], F32R, tag=f"kkT{i}", name=f"kkT{i}") for i in range(KT)]
            vv = [pp.tile([P, K], F32R, tag=f"vv{i}", name=f"vv{i}") for i in range(NT)]
            qT = [pp.tile([P, QF], F32R, tag=f"qT{i}", name=f"qT{i}") for i in range(KT)]
            retr_sb = [pp.tile([P, QF], F32R, tag=f"retr{i}", name=f"retr{i}") for i in range(KT)]
            g2T = [pp.tile([P, QF], F32, tag=f"g2T{i}", name=f"g2T{i}") for i in range(KT)]
            gn2T = [pp.tile([P, QF], F32R, tag=f"gn2T{i}", name=f"gn2T{i}") for i in range(KT)]
            h_sb = [pp.tile([P, QF], F32R, tag=f"h{n}", name=f"h{n}") for n in range(NB)]
            gates = [pp.tile([P, NB], F32, tag=f"gates{i}", name=f"gates{i}") for i in range(NQ)]
            o_sb = [pp.tile([P, K], F32, tag=f"o{i}", name=f"o{i}") for i in range(NQ)]

            # ---- helper: rmsnorm in k-major layout over a 512-wide chunk ----
            # src/dst: list of KT tiles; cols = slice of the free dim
            def rms_norm_T(src, dst, cols, w):
                sq = wp.tile([P, KT, 512], F32R, tag="sq")
                for ki in range(KT):
                    nc.vector.tensor_mul(sq[:, ki, :w], src[ki][:, cols], src[ki][:, cols])
                cs = psN.tile([1, 512], F32, tag="cs")
                for ki in range(KT):
                    nc.tensor.matmul(cs[:1, :w], ones_col[:], sq[:, ki, :w],
                                     start=(ki == 0), stop=(ki == KT - 1))
                rt = sp.tile([1, 512], F32R, tag="rt")
                nc.scalar.activation(rt[:1, :w], cs[:1, :w], AF.Sqrt,
                                     bias=eps1_t[:], scale=1.0 / K)
                bc = psN.tile([P, 512], F32, tag="bc")
                nc.tensor.matmul(bc[:, :w], ones_row[:], rt[:1, :w],
                                 start=True, stop=True)
                rinv = wp.tile([P, 512], F32, tag="rinv")
                nc.vector.reciprocal(rinv[:, :w], bc[:, :w])
                for ki in range(KT):
                    nc.vector.scalar_tensor_tensor(
                        dst[ki][:, cols], rinv[:, :w], 1.0, src[ki][:, cols],
                        op0=OP.mult, op1=OP.mult)

            # ---- phase A: ones + g data first, then params ----
            nc.sync.dma_start(ones_col[:], onesc_d[:])
            nc.sync.dma_start(ones_row[:], onesr_d[:])
            for ki in range(KT):
                nc.sync.dma_start(gT[ki][:, 0:512], gT_d[ki, :, 0:512])
            for ki in range(KT):
                nc.sync.dma_start(gqT[ki][:], gqT_d[ki])
            for jc in range(1, NC5):
                for ki in range(KT):
                    nc.sync.dma_start(gT[ki][:, jc * 512:(jc + 1) * 512],
                                      gT_d[ki, :, jc * 512:(jc + 1) * 512])
            nc.sync.dma_start(pack_t[:], pack_d[:])
            nc.sync.dma_start(biash_t[:], biash_d[:])
            nc.sync.dma_start(gateb_t[:], gateb_d[:])
            # wd prefetch: decay-weight tile pairs stream in behind the g loads
            wd2 = [wp.tile([P, 2, QF], F32, tag=f"wd2_{jp}", name=f"wd2_{jp}", bufs=1)
                   for jp in range(NT // 2)]
            for jp in range(NT // 2):
                nc.sync.dma_start(wd2[jp][:], wd2_d[jp])
            rms_norm_T(gT, gnT, slice(0, 512), 512)
            rms_norm_T(gqT, gqnT, slice(0, QF), QF)
            for jc in range(1, NC5):
                rms_norm_T(gT, gnT, slice(jc * 512, (jc + 1) * 512), 512)

            # ---- phase B: per-chunk kkT + vv so attention on early j-tiles
            # is unblocked as soon as the q path is ready; qT last ----
            for jc in range(NC5):
                for ko in range(KT):
                    ps = psM.tile([P, 512], F32, tag="mm")
                    for ki in range(KT):
                        nc.tensor.matmul(
                            ps[:], (w_t[:, WK, ki, ko * P:(ko + 1) * P]),
                            (gnT[ki][:, jc * 512:(jc + 1) * 512]),
                            start=(ki == 0), stop=(ki == KT - 1))
                    nc.scalar.copy(kkT[ko][:, jc * 512:(jc + 1) * 512], ps[:])
                for jt in range(4 * jc, 4 * jc + 4):
                    ps = psM.tile([P, K], F32, tag="mm")
                    for ki in range(KT):
                        nc.tensor.matmul(
                            ps[:], (gnT[ki][:, jt * P:(jt + 1) * P]), (w_t[:, WV, ki, :]),
                            start=(ki == 0), stop=(ki == KT - 1))
                    nc.vector.tensor_copy(vv[jt][:], ps[:])
            for ko in range(KT):
                ps = psM.tile([P, QF], F32, tag="mm")
                for ki in range(KT):
                    nc.tensor.matmul(
                        ps[:], (w_t[:, WQ, ki, ko * P:(ko + 1) * P]), (gqnT[ki][:]),
                        start=(ki == 0), stop=(ki == KT - 1))
                nc.scalar.mul(qT[ko][:], ps[:], s_qk)  # fold 1/sqrt(K)

            # ---- phase C: decayed anti-causal attention ----
            retr_ps = [psR.tile([P, QF], F32, tag=f"rps{kt}", name=f"rps{kt}")
                       for kt in range(KT)]
            for jt in range(NT):
                sc = psM.tile([P, QF], F32, tag="mm", name="sc")
                for ki in range(KT):
                    nc.tensor.matmul(
                        sc[:], (kkT[ki][:, jt * P:(jt + 1) * P]), (qT[ki][:]),
                        start=(ki == 0), stop=(ki == KT - 1))
                ws = wp.tile([P, QF], F32R, tag="ws")
                nc.vector.tensor_mul(ws[:], sc[:], wd2[jt // 2][:, jt % 2, :])
                for kt in range(KT):
                    nc.tensor.matmul(
                        retr_ps[kt][:], (vv[jt][:, kt * P:(kt + 1) * P]), (ws[:]),
                        start=(jt == 0), stop=(jt == NT - 1))
            for kt in range(KT):
                nc.vector.tensor_copy(retr_sb[kt][:], retr_ps[kt][:])

            # ---- phase D: Wo, residual, second rmsnorm (k-major) ----
            for ko in range(KT):
                ps = psM.tile([P, QF], F32, tag="mm")
                for ki in range(KT):
                    nc.tensor.matmul(
                        ps[:], (w_t[:, WO, ki, ko * P:(ko + 1) * P]), (retr_sb[ki][:]),
                        start=(ki == 0), stop=(ki == KT - 1))
                # g2T = gqT + c_mem * memT   (c_mem = out_scale * mem_scale)
                nc.vector.scalar_tensor_tensor(
                    g2T[ko][:], ps[:], c_mem, gqT[ko][:],
                    op0=OP.mult, op1=OP.add)
            rms_norm_T(g2T, gn2T, slice(0, QF), QF)

            # ---- phase E: gates + dendritic MLP ----
            for n in range(NB):
                hp = psM.tile([P, QF], F32, tag="mm")
                for ki in range(KT):
                    nc.tensor.matmul(
                        hp[:], (bd_t[:, n, ki, :]), (gn2T[ki][:]),
                        start=(ki == 0), stop=(ki == KT - 1))
                nc.scalar.activation(h_sb[n][:], hp[:], AF.Gelu, bias=biash_t[:])

            for qt in range(NQ):
                gp = psM.tile([P, NB], F32, tag="mm")
                for ki in range(KT):
                    nc.tensor.matmul(
                        gp[:], gn2T[ki][:, qt * P:(qt + 1) * P], gw_t[:, ki, :],
                        start=(ki == 0), stop=(ki == KT - 1))
                gsb = sp.tile([P, NB], F32, tag="gsb")
                nc.vector.tensor_add(gsb[:], gp[:], gateb_t[:])
                mx = sp.tile([P, 1], F32, tag="mx")
                nc.vector.reduce_max(mx[:], gsb[:], axis=AX)
                sh = sp.tile([P, NB], F32, tag="sh")
                nc.vector.tensor_scalar(sh[:], gsb[:], mx[:], None, op0=OP.subtract)
                ex = sp.tile([P, NB], F32, tag="ex")
                nc.scalar.activation(ex[:], sh[:], AF.Exp)
                sm = sp.tile([P, 1], F32, tag="sm")
                nc.vector.reduce_sum(sm[:], ex[:], axis=AX)
                rc = sp.tile([P, 1], F32, tag="rc")
                nc.vector.reciprocal(rc[:], sm[:])
                nc.vector.tensor_scalar(
                    gates[qt][:], ex[:], rc[:], s_out, op0=OP.mult, op1=OP.mult)

            for qt in range(NQ):
                for n in range(NB):
                    bp = psM.tile([P, K], F32, tag="mm")
                    nc.tensor.matmul(
                        bp[:], (h_sb[n][:, qt * P:(qt + 1) * P]), (bu_t[:, n, :]),
                        start=True, stop=True)
                    if n == 0:
                        nc.vector.tensor_scalar_mul(o_sb[qt][:], bp[:], gates[qt][:, 0:1])
                    else:
                        nc.vector.scalar_tensor_tensor(
                            o_sb[qt][:], bp[:], gates[qt][:, n:n + 1], o_sb[qt][:],
                            op0=OP.mult, op1=OP.add)
                nc.sync.dma_start(o_d[qt], o_sb[qt][:])

    nc.compile()
    return nc


def kernel(**inputs):
    x = np.asarray(inputs["x"], np.float32)
    Wq = np.asarray(inputs["Wq"], np.float32)
    Wk = np.asarray(inputs["Wk"], np.float32)
    Wv = np.asarray(inputs["Wv"], np.float32)
    Wo = np.asarray(inputs["Wo"], np.float32)
    decay_logit = np.float32(np.asarray(inputs["decay_logit"]).reshape(()))
    out_scale = np.float32(np.asarray(inputs["out_scale"]).reshape(()))
    mem_scale = np.float32(np.asarray(inputs["mem_scale"]).reshape(-1)[0])
    branch_down = np.asarray(inputs["branch_down"], np.float32)
    branch_up = np.asarray(inputs["branch_up"], np.float32)
    mlp_bias = np.asarray(inputs["mlp_bias"], np.float32)
    gate_W = np.asarray(inputs["gate_W"], np.float32)
    gate_b = np.asarray(inputs["gate_b"], np.float32)
    write_scale = np.float32(np.asarray(inputs["write_scale"]).reshape(()))
    read_idx = np.asarray(inputs["read_indices"]).astype(np.int64)
    write_idx = np.asarray(inputs["write_indices"]).astype(np.int64)

    # Host-side gather of the active vocab subspace (data movement only).
    g = np.take(x, read_idx, axis=2)  # (B, T, K)

    decay = np.float32(1.0) / (np.float32(1.0) + np.exp(-decay_logit, dtype=np.float32))

    s_qk = float(1.0 / np.sqrt(np.float32(K)))
    c_mem = float(out_scale * mem_scale)
    s_out = float(write_scale * np.float32(1.0 / 16.0))

    key = (round(s_qk, 12), round(c_mem, 12), round(s_out, 12))
    nc = _prog_cache.get(key)
    if nc is None:
        nc = _build_program(s_qk, c_mem, s_out)
        _prog_cache[key] = nc

    # Replicated parameter layouts (partition-first), packed per partition.
    wall = np.stack([Wq, Wk, Wv, Wo]).reshape(4, KT, P, K).transpose(2, 0, 1, 3)
    bdall = branch_down.reshape(NB, KT, P, INNER).transpose(2, 0, 1, 3)
    buall = branch_up.transpose(1, 0, 2)
    gw = gate_W.reshape(KT, P, NB).transpose(1, 0, 2)
    pack = np.concatenate([
        wall.reshape(P, -1), bdall.reshape(P, -1), buall.reshape(P, -1),
        gw.reshape(P, -1), np.ones((P, 1), np.float32)], axis=1).astype(np.float32)
    biash = mlp_bias.reshape(P, 1).copy()
    gateb = np.broadcast_to(gate_b, (P, NB)).copy()

    # Per-core decay-weight matrices W_T[j, i_local] = decay^(j-i-1) for j>i.
    jj = np.arange(T, dtype=np.float32)[:, None]
    gT_host = [np.ascontiguousarray(g[b].T).reshape(KT, P, T) for b in range(B)]
    in_maps = []
    for c in range(8):
        b, qc = divmod(c, NQ)
        ii = (np.arange(QF, dtype=np.float32) + qc * QF)[None, :]
        expo = np.maximum(jj - ii - np.float32(1.0), np.float32(0.0)).astype(np.float32)
        wdm = np.power(decay, expo, dtype=np.float32)
        wdm[jj <= ii] = np.float32(0.0)
        gqT_host = np.ascontiguousarray(g[b][qc * QF:(qc + 1) * QF].T).reshape(KT, P, QF)
        in_maps.append({
            "gT": gT_host[b],
            "gqT": gqT_host,
            "wd": np.ascontiguousarray(wdm.reshape(NT // 2, 2, P, QF).swapaxes(1, 2)),
            "pack": pack, "biash": biash, "gateb": gateb,
            "onesc": np.ones((P, 1), np.float32),
            "onesr": np.ones((1, P), np.float32),
        })

    res = run_bass_kernel_spmd(nc, in_maps, list(range(8)))

    out = np.zeros((B, T, V), np.float32)
    for c in range(8):
        b, qc = divmod(c, NQ)
        oc = res.results[c]["o"].reshape(QF, K)
        out[b, qc * QF:(qc + 1) * QF, :][:, write_idx] = oc
    return out



# revision 10
# speedup vs baseline: 1.5043x; 1.5043x over previous
"""Trainium2 Bass kernel for nn_ColumnStep (scatter_memory).

Contract: kernel(**inputs) takes FULL unsharded inputs (numpy-convertible),
returns the FULL (B, T, V) float32 output.

Sharding: 8 cores = B(2) x T-query-chunks(4); parameters replicated. Host
does only the vocab gather / zero-scatter and layout prep.

Key idea: decay = sigmoid(decay_logit) makes the anti-causal attention
weights decay^(j-i-1) negligible beyond a ~256-token future window
(decay^256 ~ 4e-6 at logit 3.0), so each core only loads/computes a
(512 own + 128*(ND-1) future)-column window instead of the full T=2048
sequence, and the (T x 512) decay-weight DMA collapses to one
[128, 128*ND] Toeplitz band master. Score/retrieve matmuls use bf16
moving operands (full-rate at 128-wide on the PE cost model); projections
stay float32r. All layouts are k-major so rmsnorm reductions are
ones-vector matmuls and no transposes are needed.
"""

import sys

for _p in ("/opt/trn_rl_repo", "/root/.axon_site/_ro/trn_rl_repo"):
    if _p not in sys.path:
        sys.path.append(_p)

import math

import numpy as np

import concourse.bass as bass  # noqa: F401  (registers engine mixins)
import concourse.mybir as mybir
from concourse import bacc, tile
from concourse.bass_utils import run_bass_kernel_spmd

F32 = mybir.dt.float32
F32R = mybir.dt.float32r
BF16 = mybir.dt.bfloat16
AF = mybir.ActivationFunctionType
OP = mybir.AluOpType

# Problem shape (hardcoded per spec)
V, K, B, T, NB, INNER = 32000, 256, 2, 2048, 4, 128
EPS = 1.1920929e-07
P = 128          # partitions
QF = T // 4      # 512 query rows per core
NQ = QF // P     # 4 query tiles per core
KT = K // P      # 2 tiles along the k=256 dim

# pack offsets (f32 columns per partition)
O_W = 0
O_BD = O_W + 4 * KT * K          # 2048
O_BU = O_BD + NB * KT * INNER    # 3072
O_GW = O_BU + NB * K             # 4096
PK = O_GW + KT * NB              # 4104
WK, WQ, WV, WO = 0, 1, 2, 3

_prog_cache = {}


def _build_program(s_qk, c_mem, nd):
    """SPMD Bass/Tile program. nd = number of 128-wide j-tile diagonals
    (1 own + nd-1 future) each query tile attends to."""
    nc = bacc.Bacc("TRN2", target_bir_lowering=False, debug=False, num_devices=8)

    WIN = QF + P * (nd - 1)   # key/value window columns per core
    NJ = NQ + nd - 1          # local j tiles
    MW = P * nd               # decay master columns

    gw_d = nc.dram_tensor("gw", [P, KT, WIN], F32, kind="ExternalInput")
    m_d = nc.dram_tensor("m", [P, MW + 1], F32, kind="ExternalInput")
    pack_d = nc.dram_tensor("pack", [P, PK], F32R, kind="ExternalInput")
    # small: onesc | biash | gatebT (col; rows 0..NB-1 hold gate_b)
    small_d = nc.dram_tensor("small", [P, 3], F32R, kind="ExternalInput")
    # onesr cols: [ones(P) | s_out*ones(P)]
    onesr_d = nc.dram_tensor("onesr", [1, 2 * P], F32R, kind="ExternalInput")
    # branch-selector: e[c, n*P+p] = s_out if c == n else 0
    e_d = nc.dram_tensor("esel", [NB, NB * P], F32R, kind="ExternalInput")
    o_d = nc.dram_tensor("o", [KT, P, QF], F32, kind="ExternalOutput")

    AX = mybir.AxisListType.X

    # rms chunks over the window: [(start, end), ...] in <=512 steps
    chunks = [(c, min(c + 512, WIN)) for c in range(0, WIN, 512)]

    with tile.TileContext(nc) as tc:
        with (
            nc.allow_low_precision(reason="bf16 attention operands validated by rel-err test"),
            tc.tile_pool(name="const", bufs=1) as cp,
            tc.tile_pool(name="persist", bufs=1) as pp,
            tc.tile_pool(name="work", bufs=3) as wp,
            tc.tile_pool(name="stat", bufs=4) as sp,
            tc.tile_pool(name="psA", bufs=2, space="PSUM") as psA,
            tc.tile_pool(name="psS", bufs=2, space="PSUM") as psS,
            tc.tile_pool(name="psR", bufs=1, space="PSUM") as psR,
            tc.tile_pool(name="psN", bufs=1, space="PSUM") as psN,
        ):
            # ---- constants / parameters ----
            pack_t = cp.tile([P, PK], F32R, tag="pack")
            w_t = pack_t[:, O_W:O_BD].rearrange("p (w t k) -> p w t k", w=4, t=KT)
            bd_t = pack_t[:, O_BD:O_BU].rearrange("p (n t h) -> p n t h", n=NB, t=KT)
            bu_t = pack_t[:, O_BU:O_GW].rearrange("p (n k) -> p n k", n=NB)
            gw_wt = pack_t[:, O_GW:PK].rearrange("p (t n) -> p t n", t=KT)
            small_t = cp.tile([P, 3], F32R, tag="small")
            ones_col = small_t[:, 0:1]
            biash_t = small_t[:, 1:2]
            onesr_t = cp.tile([1, 2 * P], F32R, tag="onesr")
            e_t = cp.tile([NB, NB * P], F32R, tag="esel")
            m_t = cp.tile([P, MW + 1], F32, tag="mmat")
            gatebT = m_t[:, MW:MW + 1]  # f32 column (tensor_scalar needs f32)
            eps1_t = cp.tile([1, 1], F32, tag="eps1")
            warm_t = cp.tile([1, 1], F32, tag="warm")
            gw_sb = cp.tile([P, KT, WIN], F32, tag="gwin")

            # ---- act-table warm-up (overlaps the initial DMA stall) ----
            nc.vector.memset(eps1_t[:], EPS)
            nc.vector.memset(warm_t[:], 0.0)
            nc.scalar.activation(warm_t[:], warm_t[:], AF.Sqrt)
            nc.scalar.activation(warm_t[:], warm_t[:], AF.Gelu)
            nc.scalar.activation(warm_t[:], warm_t[:], AF.Exp)

            # ---- DMAs in priority order ----
            nc.sync.dma_start(small_t[:], small_d[:])
            nc.sync.dma_start(onesr_t[:], onesr_d[:])
            nc.sync.dma_start(e_t[:], e_d[:])
            nc.sync.dma_start(gw_sb[:, :, 0:512], gw_d[:, :, 0:512])
            if WIN > 512:
                nc.sync.dma_start(gw_sb[:, :, 512:WIN], gw_d[:, :, 512:WIN])
            nc.sync.dma_start(m_t[:], m_d[:])
            nc.sync.dma_start(pack_t[:, 0:1024], pack_d[:, 0:1024])        # Wk,Wq
            nc.sync.dma_start(pack_t[:, 1024:2048], pack_d[:, 1024:2048])  # Wv,Wo
            nc.sync.dma_start(pack_t[:, 2048:3072], pack_d[:, 2048:3072])  # bd
            nc.sync.dma_start(pack_t[:, 3072:PK], pack_d[:, 3072:PK])      # bu,gw

            # ---- persistent intermediates ----
            gnT = [pp.tile([P, WIN], F32R, tag=f"gnT{i}", name=f"gnT{i}") for i in range(KT)]
            kkb = [pp.tile([P, WIN], BF16, tag=f"kkb{i}", name=f"kkb{i}") for i in range(KT)]
            qb = [pp.tile([P, QF], BF16, tag=f"qb{i}", name=f"qb{i}") for i in range(KT)]
            vvb = [pp.tile([P, K], BF16, tag=f"vvb{j}", name=f"vvb{j}") for j in range(NJ)]
            retr_sb = [pp.tile([P, QF], F32R, tag=f"retr{i}", name=f"retr{i}") for i in range(KT)]
            g2T = [pp.tile([P, QF], F32, tag=f"g2T{i}", name=f"g2T{i}") for i in range(KT)]
            gn2T = [pp.tile([P, QF], F32R, tag=f"gn2T{i}", name=f"gn2T{i}") for i in range(KT)]
            hgel = [pp.tile([P, QF], F32R, tag=f"hgel{n}", name=f"hgel{n}") for n in range(NB)]
            hg = [pp.tile([P, QF], F32R, tag=f"hg{n}", name=f"hg{n}") for n in range(NB)]
            exr = pp.tile([NB, QF], F32R, tag="exr")
            grow = pp.tile([NB, QF], F32R, tag="grow")
            rcr = pp.tile([1, QF], F32R, tag="rcr")
            o_sb = [pp.tile([P, QF], F32, tag=f"o{kt}", name=f"o{kt}") for kt in range(KT)]

            # ---- rmsnorm (k-major): reduce over partitions via ones matmul;
            # engines for the two squares / two applies are picked to run in
            # parallel (Act + DVE) ----
            def rms_norm(src, dst, c0, c1):
                w = c1 - c0
                sq = wp.tile([P, KT, 512], F32R, tag="sq")
                nc.scalar.square(sq[:, 0, :w], src(0))
                nc.vector.tensor_mul(sq[:, 1, :w], src(1), src(1))
                cs = psN.tile([1, 512], F32, tag="cs")
                for ki in range(KT):
                    nc.tensor.matmul(cs[:1, :w], ones_col, sq[:, ki, :w],
                                     start=(ki == 0), stop=(ki == KT - 1))
                rt = sp.tile([1, 512], F32R, tag="rt")
                nc.scalar.activation(rt[:1, :w], cs[:1, :w], AF.Sqrt,
                                     bias=eps1_t[:], scale=1.0 / K)
                bc = psN.tile([P, 512], F32, tag="bc")
                nc.tensor.matmul(bc[:, :w], onesr_t[:, 0:P], rt[:1, :w],
                                 start=True, stop=True)
                rinv = wp.tile([P, 512], F32, tag="rinv")
                nc.vector.reciprocal(rinv[:, :w], bc[:, :w])
                nc.vector.tensor_mul(dst[0][:, c0:c1], src(0), rinv[:, :w])
                nc.gpsimd.tensor_mul(dst[1][:, c0:c1], src(1), rinv[:, :w])

            for (c0, c1) in chunks:
                rms_norm(lambda ki, a=c0, b=c1: gw_sb[:, ki, a:b], gnT, c0, c1)

            # ---- projections: kk over window, q over own rows, vv per tile ----
            for (c0, c1) in chunks:
                w = c1 - c0
                for ko in range(KT):
                    ps = psA.tile([P, 512], F32, tag="mm")
                    for ki in range(KT):
                        nc.tensor.matmul(
                            ps[:, :w], w_t[:, WK, ki, ko * P:(ko + 1) * P],
                            gnT[ki][:, c0:c1],
                            start=(ki == 0), stop=(ki == KT - 1))
                    nc.scalar.copy(kkb[ko][:, c0:c1], ps[:, :w])
            for ko in range(KT):
                ps = psA.tile([P, 512], F32, tag="mm")
                for ki in range(KT):
                    nc.tensor.matmul(
                        ps[:], w_t[:, WQ, ki, ko * P:(ko + 1) * P],
                        gnT[ki][:, 0:QF],
                        start=(ki == 0), stop=(ki == KT - 1))
                nc.scalar.mul(qb[ko][:], ps[:], s_qk)
            for jt in range(NJ):
                ps = psA.tile([P, K], F32, tag="mm")
                for ki in range(KT):
                    nc.tensor.matmul(
                        ps[:], gnT[ki][:, jt * P:(jt + 1) * P], w_t[:, WV, ki, :],
                        start=(ki == 0), stop=(ki == KT - 1))
                nc.vector.tensor_copy(vvb[jt][:], ps[:])

            # ---- windowed anti-causal decay attention ----
            retr_ps = [psR.tile([P, QF], F32, tag=f"rps{kt}", name=f"rps{kt}") for kt in range(KT)]
            for jt in range(NJ):
                lo = max(0, jt - (nd - 1))
                hi = min(NQ - 1, jt)
                ib = lo * P
                wdt = (hi - lo + 1) * P
                ms = P * (nd - 1) - P * min(jt, nd - 1)
                sc = psS.tile([P, 512], F32, tag="sc")
                for ki in range(KT):
                    nc.tensor.matmul(
                        sc[:, :wdt], kkb[ki][:, jt * P:(jt + 1) * P],
                        qb[ki][:, ib:ib + wdt],
                        start=(ki == 0), stop=(ki == KT - 1))
                ws = wp.tile([P, 512], BF16, tag="ws")
                nc.vector.tensor_mul(ws[:, :wdt], sc[:, :wdt],
                                     m_t[:, ms:ms + wdt])
                for qt in range(lo, hi + 1):
                    off = qt * P - ib
                    for kt in range(KT):
                        nc.tensor.matmul(
                            retr_ps[kt][:, qt * P:(qt + 1) * P],
                            vvb[jt][:, kt * P:(kt + 1) * P],
                            ws[:, off:off + P],
                            start=(jt == qt), stop=(jt == qt + nd - 1))
            for kt in range(KT):
                nc.scalar.copy(retr_sb[kt][:], retr_ps[kt][:])

            # ---- Wo, residual, second rmsnorm ----
            for ko in range(KT):
                ps = psA.tile([P, 512], F32, tag="mm")
                for ki in range(KT):
                    nc.tensor.matmul(
                        ps[:], w_t[:, WO, ki, ko * P:(ko + 1) * P], retr_sb[ki][:],
                        start=(ki == 0), stop=(ki == KT - 1))
                nc.vector.scalar_tensor_tensor(
                    g2T[ko][:], ps[:], c_mem, gw_sb[:, ko, 0:QF],
                    op0=OP.mult, op1=OP.add)
            rms_norm(lambda ki: g2T[ki][:, 0:QF], gn2T, 0, QF)

            # ---- dendritic MLP down + gelu ----
            for n in range(NB):
                hp = psA.tile([P, QF], F32, tag="mm")
                for ki in range(KT):
                    nc.tensor.matmul(
                        hp[:], bd_t[:, n, ki, :], gn2T[ki][:],
                        start=(ki == 0), stop=(ki == KT - 1))
                nc.scalar.activation(hgel[n][:], hp[:], AF.Gelu, bias=biash_t)

            # ---- gates: batched row-layout softmax over NB=4 partitions ----
            gp = psS.tile([NB, QF], F32, tag="sc")
            for ki in range(KT):
                nc.tensor.matmul(gp[:], gw_wt[:, ki, :], gn2T[ki][:],
                                 start=(ki == 0), stop=(ki == KT - 1))
            gsb = sp.tile([NB, QF], F32, tag="gsb")
            nc.vector.tensor_scalar(gsb[:], gp[:], gatebT[0:NB, :], None,
                                    op0=OP.add)
            nc.scalar.activation(exr[:], gsb[:], AF.Exp)
            sm = psS.tile([1, QF], F32, tag="sc")
            nc.tensor.matmul(sm[:], ones_col[0:NB, :], exr[:],
                             start=True, stop=True)
            nc.vector.reciprocal(rcr[:], sm[:])
            rcb = psS.tile([NB, QF], F32, tag="sc")
            nc.tensor.matmul(rcb[:], onesr_t[:, 0:NB], rcr[:],
                             start=True, stop=True)
            nc.vector.tensor_mul(grow[:], exr[:], rcb[:])

            # gate the hidden branches: gb = s_out * gates_row broadcast
            for n in range(NB):
                gb = psA.tile([P, QF], F32, tag="mm")
                nc.tensor.matmul(gb[:], e_t[:, n * P:(n + 1) * P], grow[:],
                                 start=True, stop=True)
                nc.vector.tensor_mul(hg[n][:], hgel[n][:], gb[:])

            # ---- up-projection, k-major output: accumulate over branches ----
            for kt in range(KT):
                bp = psA.tile([P, QF], F32, tag="mm")
                for n in range(NB):
                    nc.tensor.matmul(
                        bp[:], bu_t[:, n, kt * P:(kt + 1) * P], hg[n][:],
                        start=(n == 0), stop=(n == NB - 1))
                nc.scalar.copy(o_sb[kt][:], bp[:])
                nc.sync.dma_start(o_d[kt], o_sb[kt][:])

    nc.compile()
    return nc


def kernel(**inputs):
    x = np.asarray(inputs["x"], np.float32)
    Wq = np.asarray(inputs["Wq"], np.float32)
    Wk = np.asarray(inputs["Wk"], np.float32)
    Wv = np.asarray(inputs["Wv"], np.float32)
    Wo = np.asarray(inputs["Wo"], np.float32)
    decay_logit = np.float32(np.asarray(inputs["decay_logit"]).reshape(()))
    out_scale = np.float32(np.asarray(inputs["out_scale"]).reshape(()))
    mem_scale = np.float32(np.asarray(inputs["mem_scale"]).reshape(-1)[0])
    branch_down = np.asarray(inputs["branch_down"], np.float32)
    branch_up = np.asarray(inputs["branch_up"], np.float32)
    mlp_bias = np.asarray(inputs["mlp_bias"], np.float32)
    gate_W = np.asarray(inputs["gate_W"], np.float32)
    gate_b = np.asarray(inputs["gate_b"], np.float32)
    write_scale = np.float32(np.asarray(inputs["write_scale"]).reshape(()))
    read_idx = np.asarray(inputs["read_indices"]).astype(np.int64)
    write_idx = np.asarray(inputs["write_indices"]).astype(np.int64)

    # Host-side gather of the active vocab subspace (data movement only).
    g = np.take(x, read_idx, axis=2)  # (B, T, K)

    decay = float(1.0 / (1.0 + np.exp(-float(decay_logit))))
    # window depth: smallest nd with decay^(128*(nd-1)) <= 3e-5 (first
    # omitted diagonal's largest weight); nd=2 minimum, 16 = full sequence
    if decay <= 0.0:
        nd = 2
    else:
        nd = max(2, 1 + int(math.ceil(math.log(3e-5) / math.log(decay) / 128.0)))
    nd = min(nd, 16)

    s_qk = float(1.0 / np.sqrt(np.float32(K)))
    c_mem = float(out_scale * mem_scale)
    s_out = float(write_scale * np.float32(1.0 / 16.0))

    key = (round(s_qk, 12), round(c_mem, 12), nd)
    nc = _prog_cache.get(key)
    if nc is None:
        nc = _build_program(s_qk, c_mem, nd)
        _prog_cache[key] = nc

    WIN = QF + P * (nd - 1)
    MW = P * nd

    # Replicated parameter pack (partition-first); wall order [Wk,Wq,Wv,Wo].
    wall = np.stack([Wk, Wq, Wv, Wo]).reshape(4, KT, P, K).transpose(2, 0, 1, 3)
    bdall = branch_down.reshape(NB, KT, P, INNER).transpose(2, 0, 1, 3)
    buall = branch_up.transpose(1, 0, 2)
    gwp = gate_W.reshape(KT, P, NB).transpose(1, 0, 2)
    pack = np.concatenate([
        wall.reshape(P, -1), bdall.reshape(P, -1), buall.reshape(P, -1),
        gwp.reshape(P, -1)], axis=1).astype(np.float32)
    small = np.zeros((P, 3), np.float32)
    small[:, 0] = 1.0
    small[:, 1] = mlp_bias
    onesr = np.ones((1, 2 * P), np.float32)
    onesr[0, P:] = s_out
    esel = np.zeros((NB, NB * P), np.float32)
    for _n in range(NB):
        esel[_n, _n * P:(_n + 1) * P] = s_out

    # Toeplitz decay master: M[jl, m] = decay^(128*(nd-1) + jl - m - 1),
    # zero where the exponent would be negative (j <= i).
    jl = np.arange(P, dtype=np.float64)[:, None]
    mm = np.arange(MW, dtype=np.float64)[None, :]
    e = P * (nd - 1) + jl - mm - 1.0
    M = np.where(e >= 0, np.power(decay, np.maximum(e, 0.0)), 0.0).astype(np.float32)
    M = np.concatenate([M, np.zeros((P, 1), np.float32)], axis=1)
    M[:NB, MW] = gate_b

    in_maps = []
    for c in range(8):
        b, qc = divmod(c, NQ)
        c0 = qc * QF
        navail = min(WIN, T - c0)
        win = np.zeros((WIN, K), np.float32)
        win[:navail] = g[b][c0:c0 + navail]
        gwc = np.ascontiguousarray(
            win.T.reshape(KT, P, WIN).transpose(1, 0, 2))
        in_maps.append({
            "gw": gwc, "m": M, "pack": pack, "small": small, "onesr": onesr,
            "esel": esel,
        })

    res = run_bass_kernel_spmd(nc, in_maps, list(range(8)))

    out = np.zeros((B, T, V), np.float32)
    for c in range(8):
        b, qc = divmod(c, NQ)
        oc = res.results[c]["o"]  # [KT, P, QF] = (k-major)^T
        ocf = oc.reshape(K, QF).T  # (QF, K)
        out[b, qc * QF:(qc + 1) * QF, :][:, write_idx] = ocf
    return out


# revision 16
# speedup vs baseline: 1.7942x; 1.1928x over previous
"""Trainium2 Bass kernel for nn_ColumnStep (scatter_memory).

Contract: kernel(**inputs) takes FULL unsharded inputs (numpy-convertible),
returns the FULL (B, T, V) float32 output.

Sharding: 8 cores = B(2) x T-query-chunks(4); parameters replicated. Host
does only the vocab gather / zero-scatter and layout prep.

Key idea: decay = sigmoid(decay_logit) makes the anti-causal attention
weights decay^(j-i-1) negligible beyond a ~256-token future window
(decay^256 ~ 4e-6 at logit 3.0), so each core only loads/computes a
(512 own + 128*(ND-1) future)-column window instead of the full T=2048
sequence, and the (T x 512) decay-weight DMA collapses to one
[128, 128*ND] Toeplitz band master. Score/retrieve matmuls use bf16
moving operands (full-rate at 128-wide on the PE cost model); projections
stay float32r. All layouts are k-major so rmsnorm reductions are
ones-vector matmuls and no transposes are needed.
"""

import sys

for _p in ("/opt/trn_rl_repo", "/root/.axon_site/_ro/trn_rl_repo"):
    if _p not in sys.path:
        sys.path.append(_p)

import math

import numpy as np

import concourse.bass as bass  # noqa: F401  (registers engine mixins)
import concourse.mybir as mybir
from concourse import bacc, tile
from concourse.bass_utils import run_bass_kernel_spmd

F32 = mybir.dt.float32
F32R = mybir.dt.float32r
BF16 = mybir.dt.bfloat16
AF = mybir.ActivationFunctionType
OP = mybir.AluOpType

# Problem shape (hardcoded per spec)
V, K, B, T, NB, INNER = 32000, 256, 2, 2048, 4, 128
EPS = 1.1920929e-07
P = 128          # partitions
QF = T // 4      # 512 query rows per core
NQ = QF // P     # 4 query tiles per core
KT = K // P      # 2 tiles along the k=256 dim

# pack offsets (f32 columns per partition)
O_W = 0
O_BD = O_W + 4 * KT * K          # 2048
O_BU = O_BD + NB * KT * INNER    # 3072
O_GW = O_BU + NB * K             # 4096
PK = O_GW + KT * NB              # 4104
WK, WQ, WV, WO = 0, 1, 2, 3

_prog_cache = {}


def _build_program(s_qk, c_mem, nd):
    """SPMD Bass/Tile program. nd = number of 128-wide j-tile diagonals
    (1 own + nd-1 future) each query tile attends to."""
    nc = bacc.Bacc("TRN2", target_bir_lowering=False, debug=False, num_devices=8)

    WIN = QF + P * (nd - 1)   # key/value window columns per core
    NJ = NQ + nd - 1          # local j tiles
    MW = P * nd               # decay master columns

    gw_d = nc.dram_tensor("gw", [P, KT, WIN], F32, kind="ExternalInput")
    m_d = nc.dram_tensor("m", [P, MW + 1], F32, kind="ExternalInput")
    pack_d = nc.dram_tensor("pack", [P, PK], F32R, kind="ExternalInput")
    # small: onesc | biash | gatebT (col; rows 0..NB-1 hold gate_b)
    small_d = nc.dram_tensor("small", [P, 3], F32R, kind="ExternalInput")
    # onesr cols: [ones(P) | s_out*ones(P)]
    onesr_d = nc.dram_tensor("onesr", [1, 2 * P], F32R, kind="ExternalInput")
    # branch-selector: e[c, n*P+p] = s_out if c == n else 0
    e_d = nc.dram_tensor("esel", [NB, NB * P], F32R, kind="ExternalInput")
    o_d = nc.dram_tensor("o", [KT, P, QF], F32, kind="ExternalOutput")

    AX = mybir.AxisListType.X

    # rms chunks over the window: [(start, end), ...] in <=512 steps
    chunks = [(c, min(c + 512, WIN)) for c in range(0, WIN, 512)]

    with tile.TileContext(nc) as tc:
        with (
            nc.allow_low_precision(reason="bf16 attention operands validated by rel-err test"),
            tc.tile_pool(name="const", bufs=1) as cp,
            tc.tile_pool(name="persist", bufs=1) as pp,
            tc.tile_pool(name="work", bufs=3) as wp,
            tc.tile_pool(name="stat", bufs=4) as sp,
            tc.tile_pool(name="psA", bufs=2, space="PSUM") as psA,
            tc.tile_pool(name="psS", bufs=2, space="PSUM") as psS,
            tc.tile_pool(name="psR", bufs=1, space="PSUM") as psR,
            tc.tile_pool(name="psN", bufs=1, space="PSUM") as psN,
        ):
            # ---- constants / parameters ----
            pack_t = cp.tile([P, PK], F32R, tag="pack")
            w_t = pack_t[:, O_W:O_BD].rearrange("p (w t k) -> p w t k", w=4, t=KT)
            bd_t = pack_t[:, O_BD:O_BU].rearrange("p (n t h) -> p n t h", n=NB, t=KT)
            bu_t = pack_t[:, O_BU:O_GW].rearrange("p (n k) -> p n k", n=NB)
            gw_wt = pack_t[:, O_GW:PK].rearrange("p (t n) -> p t n", t=KT)
            small_t = cp.tile([P, 3], F32R, tag="small")
            ones_col = small_t[:, 0:1]
            biash_t = small_t[:, 1:2]
            onesr_t = cp.tile([1, 2 * P], F32R, tag="onesr")
            e_t = cp.tile([NB, NB * P], F32R, tag="esel")
            m_t = cp.tile([P, MW + 1], F32, tag="mmat")
            gatebT = m_t[:, MW:MW + 1]  # f32 column (tensor_scalar needs f32)
            eps1_t = cp.tile([1, 1], F32, tag="eps1")
            warm_t = cp.tile([1, 1], F32, tag="warm")
            gw_sb = cp.tile([P, KT, WIN], F32, tag="gwin")

            # ---- act-table warm-up: ONE table slot — warm only Sqrt (the
            # first function used); Exp/Gelu load late, hidden behind PE ----
            nc.vector.memset(eps1_t[:], EPS)
            nc.vector.memset(warm_t[:], 0.0)
            nc.scalar.activation(warm_t[:], warm_t[:], AF.Sqrt)

            # ---- DMAs in priority order (first-use first) ----
            nc.sync.dma_start(gw_sb[:, 0, 0:512], gw_d[:, 0, 0:512])
            nc.sync.dma_start(gw_sb[:, 1, 0:512], gw_d[:, 1, 0:512])
            nc.sync.dma_start(small_t[:], small_d[:])
            nc.sync.dma_start(onesr_t[:], onesr_d[:])
            nc.sync.dma_start(pack_t[:, 0:1024], pack_d[:, 0:1024])        # Wk,Wq
            if WIN > 512:
                nc.sync.dma_start(gw_sb[:, :, 512:WIN], gw_d[:, :, 512:WIN])
            nc.sync.dma_start(m_t[:], m_d[:])
            nc.sync.dma_start(pack_t[:, 1024:2048], pack_d[:, 1024:2048])  # Wv,Wo
            nc.sync.dma_start(e_t[:], e_d[:])
            nc.sync.dma_start(pack_t[:, 2048:3072], pack_d[:, 2048:3072])  # bd
            nc.sync.dma_start(pack_t[:, 3072:PK], pack_d[:, 3072:PK])      # bu,gw

            # ---- persistent intermediates ----
            gnT = [pp.tile([P, WIN], F32R, tag=f"gnT{i}", name=f"gnT{i}") for i in range(KT)]
            kkb = [pp.tile([P, WIN], BF16, tag=f"kkb{i}", name=f"kkb{i}") for i in range(KT)]
            qb = [pp.tile([P, QF], BF16, tag=f"qb{i}", name=f"qb{i}") for i in range(KT)]
            vvb = [pp.tile([P, K], BF16, tag=f"vvb{j}", name=f"vvb{j}") for j in range(NJ)]
            retr_sb = [pp.tile([P, QF], F32R, tag=f"retr{i}", name=f"retr{i}") for i in range(KT)]
            g2T = [pp.tile([P, QF], F32, tag=f"g2T{i}", name=f"g2T{i}") for i in range(KT)]
            gn2T = [pp.tile([P, QF], F32R, tag=f"gn2T{i}", name=f"gn2T{i}") for i in range(KT)]
            hgel = [pp.tile([P, QF], F32R, tag=f"hgel{n}", name=f"hgel{n}") for n in range(NB)]
            hg = [pp.tile([P, QF], F32R, tag=f"hg{n}", name=f"hg{n}") for n in range(NB)]
            exr = pp.tile([NB, QF], F32R, tag="exr")
            rcr = pp.tile([1, QF], F32R, tag="rcr")
            o_sb = [pp.tile([P, QF], F32, tag=f"o{kt}", name=f"o{kt}") for kt in range(KT)]

            # ---- rmsnorm (k-major): reduce over partitions via ones matmul;
            # engines for the two squares / two applies are picked to run in
            # parallel (Act + DVE) ----
            def rms_norm(src, dst, c0, c1):
                w = c1 - c0
                sq = wp.tile([P, KT, 512], F32R, tag="sq")
                nc.scalar.square(sq[:, 0, :w], src(0))
                nc.vector.tensor_mul(sq[:, 1, :w], src(1), src(1))
                cs = psN.tile([1, 512], F32, tag="nrm")
                for ki in range(KT):
                    nc.tensor.matmul(cs[:1, :w], ones_col, sq[:, ki, :w],
                                     start=(ki == 0), stop=(ki == KT - 1))
                rt = sp.tile([1, 512], F32R, tag="rt")
                nc.scalar.activation(rt[:1, :w], cs[:1, :w], AF.Sqrt,
                                     bias=eps1_t[:], scale=1.0 / K)
                bc = psN.tile([P, 512], F32, tag="nrm")
                nc.tensor.matmul(bc[:, :w], onesr_t[:, 0:P], rt[:1, :w],
                                 start=True, stop=True)
                rinv = wp.tile([P, 512], F32, tag="rinv")
                nc.vector.reciprocal(rinv[:, :w], bc[:, :w])
                nc.vector.tensor_mul(dst[0][:, c0:c1], src(0), rinv[:, :w])
                nc.gpsimd.tensor_mul(dst[1][:, c0:c1], src(1), rinv[:, :w])

            # ---- pipelined: rmsnorm -> projections -> attention per 512-col
            # chunk, so early j-tiles retrieve while later chunks load ----
            retr_ps = [psR.tile([P, QF], F32, tag=f"rps{kt}", name=f"rps{kt}") for kt in range(KT)]

            def attention(jt):
                lo = max(0, jt - (nd - 1))
                hi = min(NQ - 1, jt)
                ib = lo * P
                wdt = (hi - lo + 1) * P
                ms = P * (nd - 1) - P * min(jt, nd - 1)
                sc = psS.tile([P, 512], F32, tag="sc")
                for ki in range(KT):
                    nc.tensor.matmul(
                        sc[:, :wdt], kkb[ki][:, jt * P:(jt + 1) * P],
                        qb[ki][:, ib:ib + wdt],
                        start=(ki == 0), stop=(ki == KT - 1))
                ws = wp.tile([P, 512], BF16, tag="ws")
                nc.vector.tensor_mul(ws[:, :wdt], sc[:, :wdt],
                                     m_t[:, ms:ms + wdt])
                for qt in range(lo, hi + 1):
                    off = qt * P - ib
                    for kt in range(KT):
                        nc.tensor.matmul(
                            retr_ps[kt][:, qt * P:(qt + 1) * P],
                            vvb[jt][:, kt * P:(kt + 1) * P],
                            ws[:, off:off + P],
                            start=(jt == qt), stop=(jt == qt + nd - 1))

            for ci, (c0, c1) in enumerate(chunks):
                w = c1 - c0
                rms_norm(lambda ki, a=c0, b=c1: gw_sb[:, ki, a:b], gnT, c0, c1)
                for ko in range(KT):
                    ps = psA.tile([P, 512], F32, tag="mm")
                    for ki in range(KT):
                        nc.tensor.matmul(
                            ps[:, :w], w_t[:, WK, ki, ko * P:(ko + 1) * P],
                            gnT[ki][:, c0:c1],
                            start=(ki == 0), stop=(ki == KT - 1))
                    nc.scalar.copy(kkb[ko][:, c0:c1], ps[:, :w])
                if ci == 0:
                    for ko in range(KT):
                        ps = psA.tile([P, 512], F32, tag="mm")
                        for ki in range(KT):
                            nc.tensor.matmul(
                                ps[:], w_t[:, WQ, ki, ko * P:(ko + 1) * P],
                                gnT[ki][:, 0:QF],
                                start=(ki == 0), stop=(ki == KT - 1))
                        nc.scalar.mul(qb[ko][:], ps[:], s_qk)
                for jt in range(c0 // P, min(c1 // P, NJ)):
                    ps = psA.tile([P, K], F32, tag="mm")
                    for ki in range(KT):
                        nc.tensor.matmul(
                            ps[:], gnT[ki][:, jt * P:(jt + 1) * P], w_t[:, WV, ki, :],
                            start=(ki == 0), stop=(ki == KT - 1))
                    nc.scalar.copy(vvb[jt][:], ps[:])
                for jt in range(c0 // P, min(c1 // P, NJ)):
                    attention(jt)
            for kt in range(KT):
                nc.scalar.copy(retr_sb[kt][:], retr_ps[kt][:])

            # ---- Wo, residual, second rmsnorm ----
            for ko in range(KT):
                ps = psA.tile([P, 512], F32, tag="mm")
                for ki in range(KT):
                    nc.tensor.matmul(
                        ps[:], w_t[:, WO, ki, ko * P:(ko + 1) * P], retr_sb[ki][:],
                        start=(ki == 0), stop=(ki == KT - 1))
                nc.vector.scalar_tensor_tensor(
                    g2T[ko][:], ps[:], c_mem, gw_sb[:, ko, 0:QF],
                    op0=OP.mult, op1=OP.add)
            rms_norm(lambda ki: g2T[ki][:, 0:QF], gn2T, 0, QF)

            # ---- gates first (Exp table load hides behind MLP-down PE);
            # softmax denominator is applied at the final output multiply,
            # so only unnormalized exp gates sit on the critical path ----
            gp = psS.tile([NB, QF], F32, tag="sc")
            for ki in range(KT):
                nc.tensor.matmul(gp[:], gw_wt[:, ki, :], gn2T[ki][:],
                                 start=(ki == 0), stop=(ki == KT - 1))
            # exp(logits + gate_b): gate_b folded into the activation bias
            nc.scalar.activation(exr[:], gp[:], AF.Exp, bias=gatebT[0:NB, :])
            sm = psN.tile([1, QF], F32, tag="nrm")
            nc.tensor.matmul(sm[:], ones_col[0:NB, :], exr[:],
                             start=True, stop=True)
            nc.vector.reciprocal(rcr[:], sm[:])
            rcB = psN.tile([P, QF], F32, tag="nrm")
            nc.tensor.matmul(rcB[:], onesr_t[:, 0:P], rcr[:],
                             start=True, stop=True)
            rcbs = pp.tile([P, QF], F32, tag="rcbs")
            nc.scalar.copy(rcbs[:], rcB[:])

            # ---- dendritic MLP down + gelu ----
            for n in range(NB):
                hp = psA.tile([P, QF], F32, tag="mm")
                for ki in range(KT):
                    nc.tensor.matmul(
                        hp[:], bd_t[:, n, ki, :], gn2T[ki][:],
                        start=(ki == 0), stop=(ki == KT - 1))
                nc.scalar.activation(hgel[n][:], hp[:], AF.Gelu, bias=biash_t)

            # gate branches with UNNORMALIZED s_out*exp gates (via esel rows)
            for n in range(NB):
                gb = psS.tile([P, QF], F32, tag="sc")
                nc.tensor.matmul(gb[:], e_t[:, n * P:(n + 1) * P], exr[:],
                                 start=True, stop=True)
                nc.vector.tensor_mul(hg[n][:], hgel[n][:], gb[:])

            # ---- up-projection, k-major output; branch-interleaved PSUM
            # accumulation so bp matmuls chase the hg muls ----
            bp = [psA.tile([P, QF], F32, tag="mm", name=f"bp{kt}")
                  for kt in range(KT)]
            for n in range(NB):
                for kt in range(KT):
                    nc.tensor.matmul(
                        bp[kt][:], bu_t[:, n, kt * P:(kt + 1) * P], hg[n][:],
                        start=(n == 0), stop=(n == NB - 1))
            for kt in range(KT):
                # normalize by the softmax denominator here (broadcast recip)
                nc.vector.tensor_mul(o_sb[kt][:], bp[kt][:], rcbs[:])
                nc.sync.dma_start(o_d[kt], o_sb[kt][:])

    nc.compile()
    return nc


def kernel(**inputs):
    x = np.asarray(inputs["x"], np.float32)
    Wq = np.asarray(inputs["Wq"], np.float32)
    Wk = np.asarray(inputs["Wk"], np.float32)
    Wv = np.asarray(inputs["Wv"], np.float32)
    Wo = np.asarray(inputs["Wo"], np.float32)
    decay_logit = np.float32(np.asarray(inputs["decay_logit"]).reshape(()))
    out_scale = np.float32(np.asarray(inputs["out_scale"]).reshape(()))
    mem_scale = np.float32(np.asarray(inputs["mem_scale"]).reshape(-1)[0])
    branch_down = np.asarray(inputs["branch_down"], np.float32)
    branch_up = np.asarray(inputs["branch_up"], np.float32)
    mlp_bias = np.asarray(inputs["mlp_bias"], np.float32)
    gate_W = np.asarray(inputs["gate_W"], np.float32)
    gate_b = np.asarray(inputs["gate_b"], np.float32)
    write_scale = np.float32(np.asarray(inputs["write_scale"]).reshape(()))
    read_idx = np.asarray(inputs["read_indices"]).astype(np.int64)
    write_idx = np.asarray(inputs["write_indices"]).astype(np.int64)

    # Host-side gather of the active vocab subspace (data movement only).
    g = np.take(x, read_idx, axis=2)  # (B, T, K)

    decay = float(1.0 / (1.0 + np.exp(-float(decay_logit))))
    # window depth: smallest nd with decay^(128*(nd-1)) <= 3e-5 (first
    # omitted diagonal's largest weight); nd=2 minimum, 16 = full sequence
    if decay <= 0.0:
        nd = 2
    else:
        nd = max(2, 1 + int(math.ceil(math.log(3e-5) / math.log(decay) / 128.0)))
    nd = min(nd, 16)

    s_qk = float(1.0 / np.sqrt(np.float32(K)))
    c_mem = float(out_scale * mem_scale)
    s_out = float(write_scale * np.float32(1.0 / 16.0))

    key = (round(s_qk, 12), round(c_mem, 12), nd)
    nc = _prog_cache.get(key)
    if nc is None:
        nc = _build_program(s_qk, c_mem, nd)
        _prog_cache[key] = nc

    WIN = QF + P * (nd - 1)
    MW = P * nd

    # Replicated parameter pack (partition-first); wall order [Wk,Wq,Wv,Wo].
    wall = np.stack([Wk, Wq, Wv, Wo]).reshape(4, KT, P, K).transpose(2, 0, 1, 3)
    bdall = branch_down.reshape(NB, KT, P, INNER).transpose(2, 0, 1, 3)
    buall = branch_up.transpose(1, 0, 2)
    gwp = gate_W.reshape(KT, P, NB).transpose(1, 0, 2)
    pack = np.concatenate([
        wall.reshape(P, -1), bdall.reshape(P, -1), buall.reshape(P, -1),
        gwp.reshape(P, -1)], axis=1).astype(np.float32)
    small = np.zeros((P, 3), np.float32)
    small[:, 0] = 1.0
    small[:, 1] = mlp_bias
    onesr = np.ones((1, 2 * P), np.float32)
    onesr[0, P:] = s_out
    esel = np.zeros((NB, NB * P), np.float32)
    for _n in range(NB):
        esel[_n, _n * P:(_n + 1) * P] = s_out

    # Toeplitz decay master: M[jl, m] = decay^(128*(nd-1) + jl - m - 1),
    # zero where the exponent would be negative (j <= i).
    jl = np.arange(P, dtype=np.float64)[:, None]
    mm = np.arange(MW, dtype=np.float64)[None, :]
    e = P * (nd - 1) + jl - mm - 1.0
    M = np.where(e >= 0, np.power(decay, np.maximum(e, 0.0)), 0.0).astype(np.float32)
    M = np.concatenate([M, np.zeros((P, 1), np.float32)], axis=1)
    M[:NB, MW] = gate_b

    in_maps = []
    for c in range(8):
        b, qc = divmod(c, NQ)
        c0 = qc * QF
        navail = min(WIN, T - c0)
        win = np.zeros((WIN, K), np.float32)
        win[:navail] = g[b][c0:c0 + navail]
        gwc = np.ascontiguousarray(
            win.T.reshape(KT, P, WIN).transpose(1, 0, 2))
        in_maps.append({
            "gw": gwc, "m": M, "pack": pack, "small": small, "onesr": onesr,
            "esel": esel,
        })

    res = run_bass_kernel_spmd(nc, in_maps, list(range(8)))

    out = np.zeros((B, T, V), np.float32)
    for c in range(8):
        b, qc = divmod(c, NQ)
        oc = res.results[c]["o"]  # [KT, P, QF] = (k-major)^T
        ocf = oc.reshape(K, QF).T  # (QF, K)
        out[b, qc * QF:(qc + 1) * QF, :][:, write_idx] = ocf
    return out
